# revision 17
# baseline (speedup 1.0000x reference)
"""GCN layer kernel for Trainium2: out[b] = D^-1/2 (A[b]+I) D^-1/2 H[b] B.

Data-parallel, one graph per NeuronCore, no collectives.

Host-side refactoring (all O(N^2) data prep; the device is a pure
single-pass streaming matmul):
    d    = 1/sqrt(1 + rowsum(A))           (host)
    ATs  = (D A D)^T  in bf16              (host; folds BOTH normalizations)
    h2t  = (D^2 H)^T  in bf16              (host; carries the +I self-loop term)
so the device computes
    YT_b = sum_t X_t^T @ ATs_t[:,b]  +  B^T @ h2t[:,b],   X_t = (H B) slab t
with one pass over ATs pipelined directly behind the DMA stream. bf16 halves
HBM traffic (8.4MB vs 16.8MB); rounding errors average out over the 2048-term
contraction (~4e-3 max rel vs the 2e-2 gate).

Layout/stream choices (from trace analysis):
 - ATs pre-slabbed on host to [128, 16*2048] so each chunk DMA is one
   contiguous 4-16KB run per partition (~425 GB/s sustained).
 - Everything runs on the Sync HWDGE queue; the Scalar queue measured
   ~144 GB/s vs Sync's ~425 on this platform.
 - Chunk sizes taper up [1,1,2,4,4,2] so the PE starts (and HAM-warms)
   early; slabs 14-15 arrive as 8 interleaved 128KB column chunks so the
   final matmul/evac/output tail pipelines per 512-col block instead of
   stalling ~2.5us on one big final chunk's HBM write-receipt.
 - h2t ships second-to-last; the self-loop term is the accumulation's
   stop-pass, so it adds no head latency.
 - YT lives as 4 independent one-bank PSUM tiles so each 512-column
   block's evacuation depends only on its own stop matmul.
Output leaves as bf16 [O, N]; host upcasts + transposes.
"""
import sys

sys.path.insert(0, "/opt/trn_rl_repo")

import numpy as np
import ml_dtypes

BF16 = ml_dtypes.bfloat16
B_, N_, F_, O_ = 8, 2048, 128, 128
NT = N_ // 128  # 16 slabs
CHUNKS = [1, 1, 2, 4, 4, 2]  # slabs 0-13; slabs 14-15 stream as column chunks
N_CORES = 8

_CACHE = {}
LAST_RESULTS = None


def _build_program():
    import concourse.bacc as bacc
    import concourse.tile as tile
    import concourse.mybir as mybir

    f32 = mybir.dt.float32
    bf16 = mybir.dt.bfloat16

    nc = bacc.Bacc(None, target_bir_lowering=False)
    # packed: [p, t*N_+i] = ATs[t*128+p, i]
    ATS = nc.dram_tensor("ats", [128, NT * N_], bf16, kind="ExternalInput")
    # bw | eye | ht | d2 (fp32 bits packed into 32 bf16 columns)
    HH = nc.dram_tensor("hh", [F_, 256 + N_ + 32], bf16, kind="ExternalInput")
    OT = nc.dram_tensor("ot", [O_, N_], bf16, kind="ExternalOutput")

    chunk_start = []
    s0 = 0
    for csz in CHUNKS:
        chunk_start.append(s0)
        s0 += csz

    with tile.TileContext(nc) as tc:
        with (
            tc.tile_pool(name="const", bufs=1) as cst,
            tc.tile_pool(name="achunks", bufs=1) as ach,
            tc.tile_pool(name="xpool", bufs=1) as xpl,
            tc.tile_pool(name="outp", bufs=4) as outp,
            tc.tile_pool(name="psbig", bufs=1, space="PSUM") as psb,
            tc.tile_pool(name="pssmall", bufs=2, space="PSUM") as pss,
        ):
            hh_sb = cst.tile([128, 256 + N_ + 32], bf16, tag="hh")
            nc.sync.dma_start(out=hh_sb, in_=HH[:, :])
            bw = hh_sb[:, 0:128]
            eye = hh_sb[:, 128:256]
            ht = hh_sb[:, 256 : 256 + N_]
            d2 = hh_sb[:, 256 + N_ : 256 + N_ + 32].bitcast(f32)  # [128, 16]

            # ATs chunks (slabs 0-13) on the Sync HWDGE ring in stream order,
            # then slabs 14-15 as 8 interleaved 128KB column chunks so the
            # final matmul/evac/output tail pipelines per 512-col block
            # instead of stalling ~2.5us on one big chunk's write-receipt.
            at_slab = [None] * NT
            for ci, csz in enumerate(CHUNKS):
                st = chunk_start[ci]
                t = ach.tile([128, csz * N_], bf16, tag=f"at{ci}")
                nc.sync.dma_start(out=t, in_=ATS[:, st * N_ : (st + csz) * N_])
                for sl in range(csz):
                    at_slab[st + sl] = t[:, sl * N_ : (sl + 1) * N_]
            tail = {}
            for b in range(4):
                for s in (14, 15):
                    tt = ach.tile([128, 512], bf16, tag=f"at_s{s}b{b}")
                    nc.sync.dma_start(
                        out=tt, in_=ATS[:, s * N_ + b * 512 : s * N_ + (b + 1) * 512]
                    )
                    tail[(s, b)] = tt

            # X_t = (H @ B) slab t and X2_t = d^2 ⊙ X_t, evacuated to SBUF
            # as bf16 stationaries
            xs = []
            x2s = []
            for t in range(NT):
                x_t = xpl.tile([128, O_], bf16, tag=f"x{t}")
                xs.append(x_t)
                x2_t = xpl.tile([128, O_], bf16, tag=f"x2{t}")
                x2s.append(x2_t)
            for t in range(NT):
                p_ps = pss.tile([128, O_], f32, tag="pp")
                nc.tensor.matmul(
                    p_ps, ht[:, t * 128 : (t + 1) * 128], bw, start=True, stop=True
                )
                nc.vector.tensor_copy(xs[t], p_ps)
                nc.vector.tensor_scalar_mul(x2s[t], p_ps, d2[:, t : t + 1])

            # 4 independent one-bank accumulators for YT's 512-col blocks
            yt = []
            for b in range(4):
                yt_b = psb.tile([128, 512], f32, tag=f"yt{b}")
                yt.append(yt_b)

            def identity_add(u):
                # self-loop term: yt[u//4] diagonal block += X2_u^T
                nc.tensor.matmul(
                    yt[u // 4][:, (u % 4) * 128 : (u % 4 + 1) * 128],
                    x2s[u],
                    eye,
                    start=False,
                    stop=False,
                )

            # main accumulation over slabs 0-13, one pass behind the stream;
            # each slab's self-loop identity add (+X2_t^T into YT's diagonal
            # block) follows its mains, so it adds no new dependencies
            for t in range(NT - 2):
                for b in range(4):
                    nc.tensor.matmul(
                        yt[b],
                        xs[t],
                        at_slab[t][:, b * 512 : (b + 1) * 512],
                        start=(t == 0),
                        stop=False,
                    )
                identity_add(t)
            # block-major tail: slabs 14-15 close each block (stop on 15),
            # then evac + output, pipelined against the tail column chunks
            for b in range(4):
                nc.tensor.matmul(yt[b], xs[14], tail[(14, b)], start=False, stop=False)
                if b == 3:
                    identity_add(14)
                    identity_add(15)
                nc.tensor.matmul(yt[b], xs[15], tail[(15, b)], start=False, stop=True)
                ost = outp.tile([128, 512], bf16, tag="ost")
                nc.vector.tensor_copy(ost, yt[b])
                nc.sync.dma_start(out=OT[:, b * 512 : (b + 1) * 512], in_=ost)

    nc.compile()
    return nc


def _get_program():
    if "nc" not in _CACHE:
        _CACHE["nc"] = _build_program()
    return _CACHE["nc"]


def kernel(H, A, B):
    global LAST_RESULTS
    from concourse.bass_utils import run_bass_kernel_spmd

    nc = _get_program()

    H32 = np.asarray(H, dtype=np.float32)
    A32 = np.asarray(A, dtype=np.float32)
    B16 = np.asarray(B, dtype=np.float32).astype(BF16)

    in_maps = []
    for b in range(B_):
        Ab = A32[b]
        dvec = (1.0 / np.sqrt(1.0 + Ab.sum(axis=1, dtype=np.float64))).astype(
            np.float32
        )
        ATs = (Ab * dvec[:, None] * dvec[None, :]).T  # [j, i] fp32
        ats_packed = (
            np.ascontiguousarray(ATs.reshape(NT, 128, N_).transpose(1, 0, 2))
            .reshape(128, NT * N_)
            .astype(BF16)
        )
        Hb = H32[b]
        hh = np.zeros((F_, 256 + N_ + 32), dtype=BF16)
        hh[:, 0:128] = B16
        hh[:, 128:256] = np.eye(128, dtype=np.float32).astype(BF16)
        hh[:, 256 : 256 + N_] = Hb.T.astype(BF16)
        # d^2 per node, fp32 bits packed into the last 32 bf16 columns:
        # d2f[p, t] = d[t*128+p]^2
        d2f = (dvec * dvec).reshape(NT, 128).T.astype(np.float32)
        hh[:, 256 + N_ :] = np.ascontiguousarray(d2f).view(BF16)
        in_maps.append({"ats": ats_packed, "hh": hh})

    res = run_bass_kernel_spmd(nc, in_maps, list(range(N_CORES)))
    LAST_RESULTS = res

    out = np.empty((B_, N_, O_), dtype=np.float32)
    for b in range(B_):
        out[b] = res.results[b]["ot"].astype(np.float32).T
    return out


# revision 22
# speedup vs baseline: 1.1569x; 1.1569x over previous
"""GCN layer kernel for Trainium2: out[b] = D^-1/2 (A[b]+I) D^-1/2 H[b] B.

Data-parallel, one graph per NeuronCore, no collectives.

Host-side refactoring (all O(N^2) data prep; the device is a pure
single-pass streaming matmul):
    d    = 1/sqrt(1 + rowsum(A))           (host)
    ATs  = (D A D)^T  in bf16              (host; folds BOTH normalizations)
so the device computes
    YT_b = sum_t X_t^T @ ATs_t[:,b]  +  sum_t (d^2 X_t)^T @ I_t,
    X_t = (H B) slab t
with one pass over ATs pipelined directly behind the DMA stream. bf16 halves
HBM traffic (8.4MB vs 16.8MB); rounding errors average out over the 2048-term
contraction (~4e-3 max rel vs the 2e-2 gate).

Layout/stream choices (from trace analysis):
 - ATs pre-slabbed on host to [128, 16*2048] so each chunk DMA is one
   contiguous 4-16KB run per partition (~425 GB/s sustained).
 - Everything runs on the Sync HWDGE queue; the Scalar queue measured
   ~144 GB/s vs Sync's ~425 on this platform.
 - Chunk sizes taper up [1,1,2,4,4,2] so the PE starts (and HAM-warms)
   early; slabs 14-15 arrive as 8 interleaved 128KB column chunks so the
   final matmul/evac/output tail pipelines per 512-col block instead of
   stalling ~2.5us on one big final chunk's HBM write-receipt.
 - The +I self-loop term is 16 identity matmuls fed by DVE-scaled
   X2_t = d^2 ⊙ X_t tiles, emitted lagging 4 slabs behind the mains so
   the PE never stalls on the DVE; d^2 ships as fp32 bits inside hh.
 - YT lives as 4 independent one-bank PSUM tiles so each 512-column
   block's evacuation depends only on its own stop matmul.
Output leaves as bf16 [O, N]; host upcasts + transposes.
"""
import sys

sys.path.insert(0, "/opt/trn_rl_repo")

import numpy as np
import ml_dtypes

BF16 = ml_dtypes.bfloat16
B_, N_, F_, O_ = 8, 2048, 128, 128
NT = N_ // 128  # 16 slabs
CHUNKS = [1, 1, 2, 4, 4, 2]  # slabs 0-13; slabs 14-15 stream as column chunks
N_CORES = 8

_CACHE = {}
LAST_RESULTS = None


def _build_program():
    import concourse.bacc as bacc
    import concourse.tile as tile
    import concourse.mybir as mybir

    f32 = mybir.dt.float32
    bf16 = mybir.dt.bfloat16

    nc = bacc.Bacc(None, target_bir_lowering=False)
    # packed: [p, t*N_+i] = ATs[t*128+p, i]
    ATS = nc.dram_tensor("ats", [128, NT * N_], bf16, kind="ExternalInput")
    # bw | eye | ht | d2 (fp32 bits packed into 32 bf16 columns)
    HH = nc.dram_tensor("hh", [F_, 256 + N_ + 32], bf16, kind="ExternalInput")
    OT = nc.dram_tensor("ot", [O_, N_], bf16, kind="ExternalOutput")

    chunk_start = []
    s0 = 0
    for csz in CHUNKS:
        chunk_start.append(s0)
        s0 += csz

    with tile.TileContext(nc) as tc:
        with (
            tc.tile_pool(name="const", bufs=1) as cst,
            tc.tile_pool(name="achunks", bufs=1) as ach,
            tc.tile_pool(name="xpool", bufs=1) as xpl,
            tc.tile_pool(name="outp", bufs=4) as outp,
            tc.tile_pool(name="psbig", bufs=1, space="PSUM") as psb,
            tc.tile_pool(name="pssmall", bufs=3, space="PSUM") as pss,
        ):
            hh_sb = cst.tile([128, 256 + N_ + 32], bf16, tag="hh")
            nc.sync.dma_start(out=hh_sb, in_=HH[:, :])
            bw = hh_sb[:, 0:128]
            eye = hh_sb[:, 128:256]
            ht = hh_sb[:, 256 : 256 + N_]
            d2 = hh_sb[:, 256 + N_ : 256 + N_ + 32].bitcast(f32)  # [128, 16]

            # ATs chunks (slabs 0-13) on the Sync HWDGE ring in stream order,
            # then slabs 14-15 as 8 interleaved 128KB column chunks so the
            # final matmul/evac/output tail pipelines per 512-col block
            # instead of stalling ~2.5us on one big chunk's write-receipt.
            at_slab = [None] * NT
            for ci, csz in enumerate(CHUNKS):
                st = chunk_start[ci]
                t = ach.tile([128, csz * N_], bf16, tag=f"at{ci}")
                nc.sync.dma_start(out=t, in_=ATS[:, st * N_ : (st + csz) * N_])
                for sl in range(csz):
                    at_slab[st + sl] = t[:, sl * N_ : (sl + 1) * N_]
            tail = {}
            for b in range(4):
                for s in (14, 15):
                    tt = ach.tile([128, 512], bf16, tag=f"at_s{s}b{b}")
                    nc.sync.dma_start(
                        out=tt, in_=ATS[:, s * N_ + b * 512 : s * N_ + (b + 1) * 512]
                    )
                    tail[(s, b)] = tt

            # X_t = (H @ B) slab t and X2_t = d^2 ⊙ X_t, evacuated to SBUF
            # as bf16 stationaries
            xs = []
            x2s = []
            for t in range(NT):
                x_t = xpl.tile([128, O_], bf16, tag=f"x{t}")
                xs.append(x_t)
                x2_t = xpl.tile([128, O_], bf16, tag=f"x2{t}")
                x2s.append(x2_t)
            for t in range(NT):
                p_ps = pss.tile([128, O_], f32, tag="pp")
                nc.tensor.matmul(
                    p_ps, ht[:, t * 128 : (t + 1) * 128], bw, start=True, stop=True
                )
                nc.vector.tensor_copy(xs[t], p_ps)
                nc.vector.tensor_scalar_mul(x2s[t], p_ps, d2[:, t : t + 1])

            # 4 independent one-bank accumulators for YT's 512-col blocks
            yt = []
            for b in range(4):
                yt_b = psb.tile([128, 512], f32, tag=f"yt{b}")
                yt.append(yt_b)

            def identity_add(u):
                # self-loop term: yt[u//4] diagonal block += X2_u^T
                nc.tensor.matmul(
                    yt[u // 4][:, (u % 4) * 128 : (u % 4 + 1) * 128],
                    x2s[u],
                    eye,
                    start=False,
                    stop=False,
                )

            # main accumulation over slabs 0-13, one pass behind the stream;
            # self-loop identity adds (+X2_u^T into YT's diagonal block) lag
            # 4 slabs behind the mains so the PE never stalls on the DVE
            # producing X2
            for t in range(NT - 2):
                for b in range(4):
                    nc.tensor.matmul(
                        yt[b],
                        xs[t],
                        at_slab[t][:, b * 512 : (b + 1) * 512],
                        start=(t == 0),
                        stop=False,
                    )
                if t >= 4:
                    identity_add(t - 4)
            for u in range(NT - 6, NT - 2):
                identity_add(u)
            # block-major tail: slabs 14-15 close each block (stop on 15),
            # then evac + output, pipelined against the tail column chunks
            for b in range(4):
                nc.tensor.matmul(yt[b], xs[14], tail[(14, b)], start=False, stop=False)
                if b == 3:
                    identity_add(14)
                    identity_add(15)
                nc.tensor.matmul(yt[b], xs[15], tail[(15, b)], start=False, stop=True)
                ost = outp.tile([128, 512], bf16, tag="ost")
                nc.vector.tensor_copy(ost, yt[b])
                nc.sync.dma_start(out=OT[:, b * 512 : (b + 1) * 512], in_=ost)

    nc.compile()
    return nc


def _get_program():
    if "nc" not in _CACHE:
        _CACHE["nc"] = _build_program()
    return _CACHE["nc"]


def kernel(H, A, B):
    global LAST_RESULTS
    from concourse.bass_utils import run_bass_kernel_spmd

    nc = _get_program()

    H32 = np.asarray(H, dtype=np.float32)
    A32 = np.asarray(A, dtype=np.float32)
    B16 = np.asarray(B, dtype=np.float32).astype(BF16)

    in_maps = []
    for b in range(B_):
        Ab = A32[b]
        dvec = (1.0 / np.sqrt(1.0 + Ab.sum(axis=1, dtype=np.float64))).astype(
            np.float32
        )
        ATs = (Ab * dvec[:, None] * dvec[None, :]).T  # [j, i] fp32
        ats_packed = (
            np.ascontiguousarray(ATs.reshape(NT, 128, N_).transpose(1, 0, 2))
            .reshape(128, NT * N_)
            .astype(BF16)
        )
        Hb = H32[b]
        hh = np.zeros((F_, 256 + N_ + 32), dtype=BF16)
        hh[:, 0:128] = B16
        hh[:, 128:256] = np.eye(128, dtype=np.float32).astype(BF16)
        hh[:, 256 : 256 + N_] = Hb.T.astype(BF16)
        # d^2 per node, fp32 bits packed into the last 32 bf16 columns:
        # d2f[p, t] = d[t*128+p]^2
        d2f = (dvec * dvec).reshape(NT, 128).T.astype(np.float32)
        hh[:, 256 + N_ :] = np.ascontiguousarray(d2f).view(BF16)
        in_maps.append({"ats": ats_packed, "hh": hh})

    res = run_bass_kernel_spmd(nc, in_maps, list(range(N_CORES)))
    LAST_RESULTS = res

    out = np.empty((B_, N_, O_), dtype=np.float32)
    for b in range(B_):
        out[b] = res.results[b]["ot"].astype(np.float32).T
    return out


# revision 23
# speedup vs baseline: 1.2852x; 1.1109x over previous
"""GCN layer kernel for Trainium2: out[b] = D^-1/2 (A[b]+I) D^-1/2 H[b] B.

Data-parallel, one graph per NeuronCore, no collectives.

Precision/decomposition strategy (all O(N^2)-or-less host prep; the device
is a single PE-bound streaming pass):
    A = 0.5·J + A',  A' = A - 0.5 ∈ [-0.5, 0.5)
    d = 1/sqrt(1 + rowsum(A))                       (host)
    out = D A1 D H B = D [ A' (D H B) + 0.5·J (D H B) + (D H B) ]
Device computes (one pass over the fp8 A' stream):
    Z[o,i] = sum_t X_t^T @ A'T_t[:,i]  +  B^T (dH)^T[:,i],   X_t = d ⊙ (H B)_t
Host finishes with the rank-1 mean term and the left normalization:
    w[o] = 0.5·(d @ H) @ B            (O(N F) host flops)
    out[i,o] = d_i · (Z[o,i] + w[o])

A' ships UNSCALED in fp8 E3M4 (TRN FP8_EXP3): for values in [-0.5, 0.5) it
is a uniform ~6-bit quantizer (abs step 2^-6), and the quantization noise
averages over the 2048-term contraction (~9e-3 max rel vs the 2e-2 gate).
Mean-centering halves the top-binade error; the d_i/d_j/mean factors are
recovered exactly on host/stationaries. HBM traffic: 4.2MB A' + 1.1MB H
consts = ~5.3MB/core, making the kernel PE-bound (~18us of matmul).

Layout/stream notes (from trace analysis):
 - A' pre-slabbed on host to [128, 16*2048] so each chunk DMA is one
   contiguous 2-8KB run per partition; everything on the Sync HWDGE queue
   (the Scalar queue measured ~144 GB/s vs Sync's ~425 here).
 - Chunk sizes taper up [1,1,2,4,4,4] so the PE starts early and stays
   HAM-warm; the PE (not the stream) paces the kernel.
 - The +I term (B^T (dH)^T) is the stop-pass: 4 matmuls close the 4
   independent one-bank PSUM accumulators, each block then evacuates
   (DVE cast to bf16) and DMAs out immediately.
 - X_t = d ⊙ (H B)_t on DVE; d ships as fp32 bits inside the bf16 hh.
Output leaves as bf16 [O, N]; host adds w, scales by d_i, upcasts,
transposes.
"""
import sys

sys.path.insert(0, "/opt/trn_rl_repo")

import numpy as np
import ml_dtypes

BF16 = ml_dtypes.bfloat16
FP8 = ml_dtypes.float8_e3m4
B_, N_, F_, O_ = 8, 2048, 128, 128
NT = N_ // 128  # 16 slabs
CHUNKS = [1, 1, 2, 4, 4, 4]  # slabs per DMA chunk (tapered head)
N_CORES = 8

_CACHE = {}
LAST_RESULTS = None


def _build_program():
    import concourse.bacc as bacc
    import concourse.tile as tile
    import concourse.mybir as mybir

    f32 = mybir.dt.float32
    bf16 = mybir.dt.bfloat16
    fp8 = mybir.dt.float8e3

    nc = bacc.Bacc(None, target_bir_lowering=False)
    # packed: [p, t*N_+i] = A'T[t*128+p, i], fp8 e3m4
    ATS = nc.dram_tensor("ats", [128, NT * N_], fp8, kind="ExternalInput")
    # bw | ht | dht | d (fp32 bits packed into 32 bf16 columns)
    HH = nc.dram_tensor("hh", [F_, 128 + 2 * N_ + 32], bf16, kind="ExternalInput")
    OT = nc.dram_tensor("ot", [O_, N_], bf16, kind="ExternalOutput")

    chunk_start = []
    s0 = 0
    for csz in CHUNKS:
        chunk_start.append(s0)
        s0 += csz

    with tile.TileContext(nc) as tc:
        with (
            tc.tile_pool(name="const", bufs=1) as cst,
            tc.tile_pool(name="achunks", bufs=1) as ach,
            tc.tile_pool(name="xpool", bufs=1) as xpl,
            tc.tile_pool(name="outp", bufs=4) as outp,
            tc.tile_pool(name="psbig", bufs=1, space="PSUM") as psb,
            tc.tile_pool(name="pssmall", bufs=3, space="PSUM") as pss,
        ):
            hh_sb = cst.tile([128, 128 + 2 * N_ + 32], bf16, tag="hh")
            nc.sync.dma_start(out=hh_sb, in_=HH[:, :])
            bw = hh_sb[:, 0:128]
            ht = hh_sb[:, 128 : 128 + N_]
            dht = hh_sb[:, 128 + N_ : 128 + 2 * N_]
            d_ap = hh_sb[:, 128 + 2 * N_ : 128 + 2 * N_ + 32].bitcast(f32)  # [128,16]

            # A' chunks on the Sync HWDGE ring in stream order
            at_slab = [None] * NT
            for ci, csz in enumerate(CHUNKS):
                st = chunk_start[ci]
                t = ach.tile([128, csz * N_], fp8, tag=f"at{ci}")
                nc.sync.dma_start(out=t, in_=ATS[:, st * N_ : (st + csz) * N_])
                for sl in range(csz):
                    at_slab[st + sl] = t[:, sl * N_ : (sl + 1) * N_]

            # X_t = d ⊙ (H @ B) slab t, bf16 stationaries
            xs = []
            for t in range(NT):
                x_t = xpl.tile([128, O_], bf16, tag=f"x{t}")
                xs.append(x_t)
            for t in range(NT):
                p_ps = pss.tile([128, O_], f32, tag="pp")
                nc.tensor.matmul(
                    p_ps, ht[:, t * 128 : (t + 1) * 128], bw, start=True, stop=True
                )
                nc.vector.tensor_scalar_mul(xs[t], p_ps, d_ap[:, t : t + 1])

            # 4 independent one-bank accumulators for Z^T's 512-col blocks
            yt = []
            for b in range(4):
                yt_b = psb.tile([128, 512], f32, tag=f"yt{b}")
                yt.append(yt_b)

            # main accumulation: one pass over the fp8 A' stream
            for t in range(NT):
                for b in range(4):
                    nc.tensor.matmul(
                        yt[b],
                        xs[t],
                        at_slab[t][:, b * 512 : (b + 1) * 512],
                        start=(t == 0),
                        stop=False,
                    )
            # +I self-loop term B^T (dH)^T closes each block (stop=True),
            # then evacuate + write out as bf16
            for b in range(4):
                nc.tensor.matmul(
                    yt[b],
                    bw,
                    dht[:, b * 512 : (b + 1) * 512],
                    start=False,
                    stop=True,
                )
                ost = outp.tile([128, 512], bf16, tag="ost")
                nc.vector.tensor_copy(ost, yt[b])
                nc.sync.dma_start(out=OT[:, b * 512 : (b + 1) * 512], in_=ost)

    nc.compile()
    return nc


def _get_program():
    if "nc" not in _CACHE:
        _CACHE["nc"] = _build_program()
    return _CACHE["nc"]


def kernel(H, A, B):
    global LAST_RESULTS
    from concourse.bass_utils import run_bass_kernel_spmd

    nc = _get_program()

    H32 = np.asarray(H, dtype=np.float32)
    A32 = np.asarray(A, dtype=np.float32)
    B32 = np.asarray(B, dtype=np.float32)

    in_maps = []
    ws = []
    ds = []
    for b in range(B_):
        Ab = A32[b]
        dvec = (1.0 / np.sqrt(1.0 + Ab.sum(axis=1, dtype=np.float64))).astype(
            np.float32
        )
        ds.append(dvec)
        # centered, unscaled A' in fp8 e3m4, slab-packed transposed
        ats_packed = (
            np.ascontiguousarray((Ab - 0.5).T.reshape(NT, 128, N_).transpose(1, 0, 2))
            .reshape(128, NT * N_)
            .astype(FP8)
        )
        Hb = H32[b]
        hh = np.zeros((F_, 128 + 2 * N_ + 32), dtype=BF16)
        hh[:, 0:128] = B32.astype(BF16)
        hh[:, 128 : 128 + N_] = Hb.T.astype(BF16)
        hh[:, 128 + N_ : 128 + 2 * N_] = (Hb * dvec[:, None]).T.astype(BF16)
        # d per node, fp32 bits in the last 32 bf16 cols: d_ap[p, t] = d[t*128+p]
        df = dvec.reshape(NT, 128).T.astype(np.float32)
        hh[:, 128 + 2 * N_ :] = np.ascontiguousarray(df).view(BF16)
        in_maps.append({"ats": ats_packed, "hh": hh})
        # host rank-1 mean term: w[o] = 0.5 * (d @ H) @ B
        ws.append(0.5 * (dvec.astype(np.float64) @ Hb.astype(np.float64)) @ B32)

    res = run_bass_kernel_spmd(nc, in_maps, list(range(N_CORES)))
    LAST_RESULTS = res

    out = np.empty((B_, N_, O_), dtype=np.float32)
    for b in range(B_):
        zt = res.results[b]["ot"].astype(np.float32)  # [O, N]
        out[b] = (zt + ws[b].astype(np.float32)[:, None]).T * ds[b][:, None]
    return out
